# revision 1
# baseline (speedup 1.0000x reference)
"""CASVDDenseMul fused kernel for 8 Trainium2 NeuronCores.

Reference computation (fp32):
    chi = sigmoid(context @ W + B)          # [B, R]
    t   = (inputs @ U) * (S * chi)          # [B, R]
    out = relu(t @ V.T + 2*bias)            # [B, UNITS]

Sharding: data-parallel over batch; each of the 8 cores handles 512 rows.
All factor weights (U, S, V, W, B, bias) are replicated.

Layout choices (all host-side, free):
  - x and context are fed transposed ([feature, batch]) so the contraction
    dim lands on SBUF partitions with no on-device transposes.
  - V is fed transposed ([R, UNITS]) so it can act as the matmul stationary
    operand per m-tile.
  - The kernel computes out.T [UNITS, batch] per core (bias is then a
    per-partition scalar, fusing bias+relu into one scalar-engine op);
    the host transposes back.
  - All matmul operands are pre-rounded to the fp32r grid (11-bit
    mantissa) on the host; the PE consumes fp32r at twice the fp32 rate.
  - U and xT are packed into one DRAM tensor ("ux"), W and ctxT into
    another ("wctx"), so each contraction chunk arrives with a single
    DMA (the Sync engine's ~0.6us per-DMA issue cost otherwise paces
    the whole input stream below HBM rate).
"""

import numpy as np

from concourse import bacc, mybir
from concourse import tile
from concourse.bass_utils import run_bass_kernel_spmd

N_CORES = 8
B_SZ, N_IN, N_CTX, UNITS, RANK = 4096, 4096, 512, 4096, 256
BS = B_SZ // N_CORES  # 512 batch rows per core

P = 128
KC_IN = N_IN // P    # 32 contraction chunks for x @ U
KC_CTX = N_CTX // P  # 4  contraction chunks for ctx @ W
RT = RANK // P       # 2  rank tiles
MT = UNITS // P      # 32 unit (output) tiles
UXW = RANK + BS      # 768 packed columns per k-chunk
# k-chunk DMA groups: big groups early (cheap issue), single chunks at the
# tail so the PE's last mm1 steps aren't gated on a wide transfer.
UX_GROUPS = [[k, k + 1] for k in range(0, 28, 2)] + [[28], [29], [30], [31]]

FP32 = mybir.dt.float32
FP32R = mybir.dt.float32r


def _build_nc():
    nc = bacc.Bacc("TRN2", target_bir_lowering=False, debug=False, enable_asserts=False)

    ux = nc.declare_dram_parameter("ux", [KC_IN, P, UXW], FP32R, isOutput=False)
    wctx = nc.declare_dram_parameter("wctx", [P, KC_CTX, UXW], FP32R, isOutput=False)
    VT = nc.declare_dram_parameter("VT", [RANK, UNITS], FP32R, isOutput=False)
    consts = nc.declare_dram_parameter("consts", [P, 2 * RT + MT], FP32, isOutput=False)
    outT = nc.declare_dram_parameter("outT", [UNITS, BS], FP32, isOutput=True)

    out_tiles = outT.rearrange("(m p) b -> m p b", p=P)   # [32, 128, 512]

    with tile.TileContext(nc) as tc:
        with (
            tc.tile_pool(name="cpool", bufs=1) as cpool,
            tc.tile_pool(name="small", bufs=1) as small,
            tc.tile_pool(name="stream", bufs=1) as stream,
            tc.tile_pool(name="acts", bufs=1) as acts,
            tc.tile_pool(name="ostage", bufs=8) as ostage,
            tc.tile_pool(name="pchi", bufs=2, space="PSUM") as pchi,
            tc.tile_pool(name="pt", bufs=1, space="PSUM") as pt,
            tc.tile_pool(name="pout", bufs=4, space="PSUM") as pout,
        ):
            # ---- small weights + constants ----
            wctx_sb = small.tile([P, KC_CTX, UXW], FP32R, tag="wctx")
            for k in range(KC_CTX):
                nc.sync.dma_start(wctx_sb[:, k, :], wctx[:, k, :])
            c_sb = cpool.tile([P, 2 * RT + MT], FP32, tag="consts")
            nc.sync.dma_start(c_sb[:], consts[:])
            s2_sb = c_sb[:, 0:RT]
            b2_sb = c_sb[:, RT:2 * RT]
            bias_sb = c_sb[:, 2 * RT:]

            # ---- PE warm-up: the HAM clock gate keeps the PE at 1.2 GHz
            # until it has been busy ~3.4us. Junk matmuls on a memset tile
            # during the (otherwise idle) DMA prologue flip it to 2.4 GHz
            # before the real stream begins.
            junk = acts.tile([P, BS], FP32, tag="junk")
            nc.gpsimd.memset(junk[:], 0.0)
            warm_ps = pchi.tile([P, BS], FP32, tag="chi", name="warm_ps")
            for _ in range(3):
                nc.tensor.matmul(
                    warm_ps[:],
                    junk[:, :P],
                    junk[:],
                    start=True,
                    stop=True,
                    skip_group_check=True,
                )

            # chi.T = sigmoid(W.T @ ctxT + B) * S -- emitted interleaved
            # with the mm1 stream below so its matmuls fill the PE's
            # DMA-wait gaps instead of delaying mm1's start.
            s_chi = [acts.tile([P, BS], FP32, tag=f"schi{rt}", name=f"schi{rt}")
                     for rt in range(RT)]

            def emit_chi(rt):
                psum_chi = pchi.tile([P, BS], FP32, tag="chi", name="psum_chi")
                for k in range(KC_CTX):
                    nc.tensor.matmul(
                        psum_chi[:],
                        wctx_sb[:, k, rt * P:(rt + 1) * P],
                        wctx_sb[:, k, RANK:],
                        start=(k == 0),
                        stop=(k == KC_CTX - 1),
                        skip_group_check=True,
                    )
                nc.scalar.activation(
                    s_chi[rt][:], psum_chi[:],
                    mybir.ActivationFunctionType.Sigmoid,
                    bias=b2_sb[:, rt:rt + 1], scale=1.0,
                )
                nc.vector.tensor_scalar_mul(
                    s_chi[rt][:], s_chi[rt][:], s2_sb[:, rt:rt + 1]
                )

            # ---- t.T = (U.T @ xT) * s_chi   (stream packed u/x groups) ----
            # VT rides the scalar HWDGE ring from the start (it is only
            # needed ~45us in, and keeping it out of the stream tail
            # stops the final x chunks crawling when the paired core's
            # stream overlaps); the u/x groups alternate rings behind it.
            ux_tiles = [stream.tile([P, len(ks), UXW], FP32R,
                                    tag=f"ux{g}", name=f"ux{g}")
                        for g, ks in enumerate(UX_GROUPS)]
            vt_sb = small.tile([P, RT, UNITS], FP32R, tag="vt")
            for c in range(RT):
                nc.scalar.dma_start(
                    vt_sb[:, c, :],
                    VT.rearrange("(c p) m -> c p m", p=P)[c],
                )
            # chi's sigmoids sit on the Scalar queue BEFORE the ux DMA
            # issues: the Scalar HWDGE ring is busy streaming VT then, so
            # the wait is free, and it keeps the chi->t' chain off the
            # critical path (otherwise sigmoid lands after ~50us of DMA
            # issues and delays mm2's start by ~3us).
            emit_chi(0)
            emit_chi(1)

            for g, ks in enumerate(UX_GROUPS):
                eng = nc.sync if g % 2 == 0 else nc.scalar
                eng.dma_start(
                    ux_tiles[g][:],
                    ux[ks[0]:ks[0] + len(ks)].rearrange("k p w -> p k w"),
                )

            psum_t = [pt.tile([P, BS], FP32, tag=f"t{rt}", name=f"pt{rt}")
                      for rt in range(RT)]
            HB = BS // 2
            for g, ks in enumerate(UX_GROUPS):
                for j, k in enumerate(ks):
                    if k == KC_IN - 1:
                        # final accumulation step in column halves, h0 first:
                        # t'-h0 (and the fast-start out tile) can begin while
                        # the h1 closing matmuls still stream.
                        for h in range(2):
                            for rt in range(RT):
                                nc.tensor.matmul(
                                    psum_t[rt][:, h * HB:(h + 1) * HB],
                                    ux_tiles[g][:, j, rt * P:(rt + 1) * P],
                                    ux_tiles[g][:, j, RANK + h * HB:RANK + (h + 1) * HB],
                                    start=False,
                                    stop=True,
                                    skip_group_check=True,
                                )
                    else:
                        for rt in range(RT):
                            nc.tensor.matmul(
                                psum_t[rt][:],
                                ux_tiles[g][:, j, rt * P:(rt + 1) * P],
                                ux_tiles[g][:, j, RANK:],
                                start=(k == 0),
                                stop=False,
                                skip_group_check=True,
                            )

            # t' in column halves: both rank-tiles' first halves are ready
            # after two 345ns DVE ops, so mm2's first tile can start ~0.7us
            # earlier than with full-width multiplies.
            H = BS // 2
            t_sb = [acts.tile([P, BS], FP32R, tag=f"t_sb{rt}", name=f"t_sb{rt}")
                    for rt in range(RT)]
            for h in range(2):
                for rt in range(RT):
                    sl = slice(h * H, (h + 1) * H)
                    nc.vector.tensor_mul(
                        t_sb[rt][:, sl], psum_t[rt][:, sl], s_chi[rt][:, sl]
                    )

            # ---- out.T = relu(V @ t.T + 2*bias) per 128-unit tile ----
            # fast-start: the first unit-tile runs at half batch width so its
            # matmuls/evac/DMA begin as soon as the first t' halves land
            for h in range(2):
                sl = slice(h * H, (h + 1) * H)
                ps0 = pout.tile([P, H], FP32, tag="o", name=f"po0h{h}")
                for c in range(RT):
                    nc.tensor.matmul(
                        ps0[:],
                        vt_sb[:, c, 0:P],
                        t_sb[c][:, sl],
                        start=(c == 0),
                        stop=(c == RT - 1),
                        skip_group_check=True,
                    )
                o0 = ostage.tile([P, H], FP32, tag="osb0", name=f"ot0h{h}")
                if h == 0:
                    nc.scalar.activation(
                        o0[:], ps0[:],
                        mybir.ActivationFunctionType.Relu,
                        bias=bias_sb[:, 0:1], scale=1.0,
                    )
                else:
                    nc.vector.tensor_scalar(
                        o0[:], ps0[:],
                        bias_sb[:, 0:1], 0.0,
                        op0=mybir.AluOpType.add, op1=mybir.AluOpType.max,
                    )
                eng = nc.sync if h == 0 else nc.scalar
                eng.dma_start(out_tiles[0][:, sl], o0[:])

            for m in range(1, MT):
                psum_o = pout.tile([P, BS], FP32, tag="o")
                for c in range(RT):
                    nc.tensor.matmul(
                        psum_o[:],
                        vt_sb[:, c, m * P:(m + 1) * P],
                        t_sb[c][:],
                        start=(c == 0),
                        stop=(c == RT - 1),
                    )
                o_sb = ostage.tile([P, BS], FP32, tag="osb")
                if m % 2 == 0:
                    nc.scalar.activation(
                        o_sb[:], psum_o[:],
                        mybir.ActivationFunctionType.Relu,
                        bias=bias_sb[:, m:m + 1], scale=1.0,
                    )
                else:
                    # split PSUM-evacuation load between ACT and DVE
                    nc.vector.tensor_scalar(
                        o_sb[:], psum_o[:],
                        bias_sb[:, m:m + 1], 0.0,
                        op0=mybir.AluOpType.add, op1=mybir.AluOpType.max,
                    )
                # alternate the two HWDGE rings so out-DMA issue keeps up
                eng = nc.sync if m % 2 == 0 else nc.scalar
                eng.dma_start(out_tiles[m], o_sb[:])

    nc.finalize()
    return nc


_NC_CACHE = {}


def _get_nc():
    if "nc" not in _NC_CACHE:
        _NC_CACHE["nc"] = _build_nc()
    return _NC_CACHE["nc"]


def _round_fp32r(a):
    """Round fp32 to the fp32r grid (11-bit mantissa; low 12 bits zero).

    The PE reads only the top 20 bits of each fp32r word; pre-rounding on
    the host gives round-to-nearest instead of hardware truncation.
    """
    u = np.ascontiguousarray(a, dtype=np.float32).view(np.uint32)
    r = (u + np.uint32(0x7FF) + ((u >> np.uint32(12)) & np.uint32(1))) & np.uint32(0xFFFFF000)
    return r.view(np.float32)


def _prepare_in_maps(inputs, context, U, S, V, W, B, bias):
    inputs = np.asarray(inputs, dtype=np.float32)
    context = np.asarray(context, dtype=np.float32)

    xTr = _round_fp32r(inputs.T)                 # [N_IN, B]
    ctxTr = _round_fp32r(context.T)              # [N_CTX, B]
    Ur = _round_fp32r(np.asarray(U, dtype=np.float32))
    Wr = _round_fp32r(np.asarray(W, dtype=np.float32))
    VTr = _round_fp32r(np.asarray(V, dtype=np.float32).T)  # [R, UNITS]

    S2 = np.asarray(S, dtype=np.float32).reshape(RT, P).T
    B2 = np.asarray(B, dtype=np.float32).reshape(RT, P).T
    bias2 = (2.0 * np.asarray(bias, dtype=np.float32)).reshape(MT, P).T
    consts = np.ascontiguousarray(
        np.concatenate([S2, B2, bias2], axis=1)
    )  # [128, 2+2+32]

    # U chunks, shared across cores: [KC_IN, P, RANK]
    u_g = Ur.reshape(KC_IN, P, RANK)
    # W chunks: [P, KC_CTX, RANK]
    w_g = Wr.reshape(KC_CTX, P, RANK).transpose(1, 0, 2)

    in_maps = []
    for c in range(N_CORES):
        sl = slice(c * BS, (c + 1) * BS)
        x_g = xTr[:, sl].reshape(KC_IN, P, BS)
        ux = np.empty((KC_IN, P, UXW), dtype=np.float32)
        ux[:, :, :RANK] = u_g
        ux[:, :, RANK:] = x_g
        ctx_g = ctxTr[:, sl].reshape(KC_CTX, P, BS).transpose(1, 0, 2)
        wctx = np.empty((P, KC_CTX, UXW), dtype=np.float32)
        wctx[:, :, :RANK] = w_g
        wctx[:, :, RANK:] = ctx_g
        in_maps.append({
            "ux": ux,
            "wctx": wctx,
            "VT": VTr,
            "consts": consts,
        })
    return in_maps


def _gather_out(results):
    out = np.empty((B_SZ, UNITS), dtype=np.float32)
    for c in range(N_CORES):
        out[c * BS:(c + 1) * BS, :] = results[c]["outT"].T
    return out


def kernel(inputs, context, U, S, V, W, B, bias):
    in_maps = _prepare_in_maps(inputs, context, U, S, V, W, B, bias)
    nc = _get_nc()
    res = run_bass_kernel_spmd(nc, in_maps, list(range(N_CORES)))
    return _gather_out(res.results)



# revision 3
# speedup vs baseline: 1.2964x; 1.2964x over previous
"""CASVDDenseMul fused kernel for 8 Trainium2 NeuronCores.

Reference computation (fp32):
    chi = sigmoid(context @ W + B)          # [B, R]
    t   = (inputs @ U) * (S * chi)          # [B, R]
    out = relu(t @ V.T + 2*bias)            # [B, UNITS]

Sharding: data-parallel over batch; each of the 8 cores handles 512 rows.
All factor weights (U, S, V, W, B, bias) are replicated.

The kernel is HBM-bandwidth-bound (~358 GB/s per core). Everything the PE
touches is fp16 (PE runs 16-bit at 1 row/cycle, same as fp32r, and the
2e-2 rel-err budget dwarfs fp16's ~4e-4), which halves HBM traffic vs the
fp32 baseline: ~13.4 MB/core total -> ~40 us roofline.

Layout choices (all host-side, free):
  - x and context are fed transposed ([feature, batch]) so the contraction
    dim lands on SBUF partitions with no on-device transposes.
  - The kernel computes out.T [UNITS, batch] per core (bias is then a
    per-partition scalar, fusing bias+relu into one scalar-engine op);
    the host transposes back.
  - U and xT are packed per 2-chunk pair into one DRAM tensor ("ux") so
    each contraction pair arrives with a single DMA of 3 KB/partition rows
    (the HWDGE's ~0.6us per-DMA issue cost otherwise paces the stream).
  - V.T is packed in 8 unit-major groups ("vt") and its DMAs are queued on
    the scalar ring BEHIND the ux odd pairs: within a ring transfers run
    in order, so VT only spends HBM bandwidth after the x stream is done,
    landing just in time for mm2.
  - The output is written as fp16 pairs of 128-unit tiles ([16,128,2,512])
    so each out DMA moves 2 KB/partition; host decodes + casts to fp32.
"""

import numpy as np

from concourse import bacc, mybir
from concourse import tile
from concourse.bass_utils import run_bass_kernel_spmd

N_CORES = 8
B_SZ, N_IN, N_CTX, UNITS, RANK = 4096, 4096, 512, 4096, 256
BS = B_SZ // N_CORES  # 512 batch rows per core

P = 128
KC_CTX = N_CTX // P      # 4 contraction chunks for ctx @ W
RT = RANK // P           # 2 rank tiles
MT = UNITS // P          # 32 unit (output) tiles
PAIRS = N_IN // (2 * P)  # 16 packed x/U chunk pairs
MG = 8                   # vt groups (512 units each)
MW = UNITS // MG         # 512
OG = MT // 2             # 16 output tile pairs
UXW = RANK + BS          # 768 packed columns per k-chunk

FP32 = mybir.dt.float32
FP16 = mybir.dt.float16


def _build_nc():
    nc = bacc.Bacc("TRN2", target_bir_lowering=False, debug=False, enable_asserts=False)

    ux = nc.declare_dram_parameter("ux", [PAIRS, P, 2, UXW], FP16, isOutput=False)
    wctx = nc.declare_dram_parameter("wctx", [P, KC_CTX, UXW], FP16, isOutput=False)
    vt = nc.declare_dram_parameter("vt", [MG, P, RT, MW], FP16, isOutput=False)
    consts = nc.declare_dram_parameter("consts", [P, 2 * RT + MT], FP32, isOutput=False)
    outT = nc.declare_dram_parameter("outT", [OG, P, 2, BS], FP16, isOutput=True)

    with tile.TileContext(nc) as tc:
        with (
            tc.tile_pool(name="cpool", bufs=1) as cpool,
            tc.tile_pool(name="small", bufs=1) as small,
            tc.tile_pool(name="stream", bufs=1) as stream,
            tc.tile_pool(name="acts", bufs=1) as acts,
            tc.tile_pool(name="ostage", bufs=10) as ostage,
            tc.tile_pool(name="pchi", bufs=2, space="PSUM") as pchi,
            tc.tile_pool(name="pt", bufs=1, space="PSUM") as pt,
            tc.tile_pool(name="pout", bufs=2, space="PSUM") as pout,
        ):
            # ---- constants + small weights ----
            c_sb = cpool.tile([P, 2 * RT + MT], FP32, tag="consts")
            nc.sync.dma_start(c_sb[:], consts[:])
            s2_sb = c_sb[:, 0:RT]
            b2_sb = c_sb[:, RT:2 * RT]
            bias_sb = c_sb[:, 2 * RT:]

            wctx_sb = small.tile([P, KC_CTX, UXW], FP16, tag="wctx")
            nc.scalar.dma_start(wctx_sb[:], wctx[:])

            # ---- PE warm-up: the HAM clock gate keeps the PE at 1.2 GHz
            # until it has been busy ~3.4us. Junk matmuls on a memset tile
            # during the (otherwise idle) DMA prologue flip it to 2.4 GHz
            # before the real stream begins.
            junk = acts.tile([P, BS], FP32, tag="junk")
            nc.gpsimd.memset(junk[:], 0.0)
            warm_ps = pchi.tile([P, BS], FP32, tag="chi", name="warm_ps")
            for _ in range(3):
                nc.tensor.matmul(
                    warm_ps[:],
                    junk[:, :P],
                    junk[:],
                    start=True,
                    stop=True,
                    skip_group_check=True,
                )

            # chi.T pre-activation = W.T @ ctxT, on the PE right behind the
            # warm-up (wctx is the first scalar-ring transfer, so it's there).
            psum_chi = [pchi.tile([P, BS], FP32, tag="chi", name=f"pchi{rt}")
                        for rt in range(RT)]
            for rt in range(RT):
                for k in range(KC_CTX):
                    nc.tensor.matmul(
                        psum_chi[rt][:],
                        wctx_sb[:, k, rt * P:(rt + 1) * P],
                        wctx_sb[:, k, RANK:],
                        start=(k == 0),
                        stop=(k == KC_CTX - 1),
                        skip_group_check=True,
                    )

            # ---- input stream: ux pairs alternate the two HWDGE rings;
            # vt rides the scalar ring BEHIND the odd pairs so its bytes
            # spend HBM bandwidth only after the x stream.
            ux_sb = [stream.tile([P, 2, UXW], FP16, tag=f"ux{g}", name=f"ux{g}")
                     for g in range(PAIRS)]
            for g in range(PAIRS):
                eng = nc.sync if g % 2 == 0 else nc.scalar
                eng.dma_start(ux_sb[g][:], ux[g])
            vt_sb = [small.tile([P, RT, MW], FP16, tag=f"vt{g}", name=f"vt{g}")
                     for g in range(MG)]
            for g in range(MG):
                nc.scalar.dma_start(vt_sb[g][:], vt[g])

            # chi epilogue: sigmoid(+B) on ACT, then *S on DVE. Queued after
            # the scalar ring's DMA issues; psum_chi is long done by then.
            s_chi = [acts.tile([P, BS], FP32, tag=f"schi{rt}", name=f"schi{rt}")
                     for rt in range(RT)]
            for rt in range(RT):
                nc.scalar.activation(
                    s_chi[rt][:], psum_chi[rt][:],
                    mybir.ActivationFunctionType.Sigmoid,
                    bias=b2_sb[:, rt:rt + 1], scale=1.0,
                )
                nc.vector.tensor_scalar_mul(
                    s_chi[rt][:], s_chi[rt][:], s2_sb[:, rt:rt + 1]
                )

            # ---- t.T = (U.T @ xT): stream the packed u/x pairs ----
            psum_t = [pt.tile([P, BS], FP32, tag=f"t{rt}", name=f"pt{rt}")
                      for rt in range(RT)]
            for g in range(PAIRS):
                for j in range(2):
                    k = 2 * g + j
                    for rt in range(RT):
                        nc.tensor.matmul(
                            psum_t[rt][:],
                            ux_sb[g][:, j, rt * P:(rt + 1) * P],
                            ux_sb[g][:, j, RANK:],
                            start=(k == 0),
                            stop=(k == 2 * PAIRS - 1),
                            skip_group_check=True,
                        )

            # t' = t * (S*chi), cast to fp16 for mm2
            t_sb = [acts.tile([P, BS], FP16, tag=f"t{rt}", name=f"t_sb{rt}")
                    for rt in range(RT)]
            for rt in range(RT):
                nc.vector.tensor_mul(t_sb[rt][:], psum_t[rt][:], s_chi[rt][:])

            # ---- out.T = relu(V @ t.T + 2*bias), two 128-unit tiles per
            # PSUM pair; ACT and DVE split the evacuation; one out DMA per
            # pair on the (otherwise idle) sync ring.
            for og in range(OG):
                po = pout.tile([P, 2, BS], FP32, tag="o", name=f"po{og}")
                osb = ostage.tile([P, 2, BS], FP16, tag="osb", name=f"osb{og}")
                for j in range(2):
                    m = 2 * og + j
                    vg, off = divmod(m, MT // MG)
                    for c in range(RT):
                        nc.tensor.matmul(
                            po[:, j, :],
                            vt_sb[vg][:, c, off * P:(off + 1) * P],
                            t_sb[c][:],
                            start=(c == 0),
                            stop=(c == RT - 1),
                            skip_group_check=True,
                        )
                    if j == 0:
                        nc.scalar.activation(
                            osb[:, 0, :], po[:, 0, :],
                            mybir.ActivationFunctionType.Relu,
                            bias=bias_sb[:, m:m + 1], scale=1.0,
                        )
                    else:
                        nc.vector.tensor_scalar(
                            osb[:, 1, :], po[:, 1, :],
                            bias_sb[:, m:m + 1], 0.0,
                            op0=mybir.AluOpType.add, op1=mybir.AluOpType.max,
                        )
                nc.sync.dma_start(outT[og], osb[:])

    nc.finalize()
    return nc


_NC_CACHE = {}


def _get_nc():
    if "nc" not in _NC_CACHE:
        _NC_CACHE["nc"] = _build_nc()
    return _NC_CACHE["nc"]


def _prepare_in_maps(inputs, context, U, S, V, W, B, bias):
    f16 = np.float16
    xT = np.ascontiguousarray(np.asarray(inputs, dtype=np.float32).T).astype(f16)
    ctxT = np.ascontiguousarray(np.asarray(context, dtype=np.float32).T).astype(f16)

    # U pairs, shared: [PAIRS, P, 2, RANK]
    u4 = np.asarray(U, dtype=np.float32).astype(f16) \
        .reshape(PAIRS, 2, P, RANK).transpose(0, 2, 1, 3)
    # W chunks, shared: [P, KC_CTX, RANK]
    w3 = np.asarray(W, dtype=np.float32).astype(f16) \
        .reshape(KC_CTX, P, RANK).transpose(1, 0, 2)
    # V.T groups, shared: [MG, P, RT, MW]
    vt4 = np.ascontiguousarray(np.asarray(V, dtype=np.float32).T).astype(f16) \
        .reshape(RT, P, MG, MW).transpose(2, 1, 0, 3)
    vt4 = np.ascontiguousarray(vt4)

    S2 = np.asarray(S, dtype=np.float32).reshape(RT, P).T
    B2 = np.asarray(B, dtype=np.float32).reshape(RT, P).T
    bias2 = (2.0 * np.asarray(bias, dtype=np.float32)).reshape(MT, P).T
    consts = np.ascontiguousarray(np.concatenate([S2, B2, bias2], axis=1))

    in_maps = []
    for c in range(N_CORES):
        sl = slice(c * BS, (c + 1) * BS)
        x4 = xT[:, sl].reshape(PAIRS, 2, P, BS).transpose(0, 2, 1, 3)
        ux = np.empty((PAIRS, P, 2, UXW), dtype=f16)
        ux[..., :RANK] = u4
        ux[..., RANK:] = x4
        ctx3 = ctxT[:, sl].reshape(KC_CTX, P, BS).transpose(1, 0, 2)
        wctx = np.empty((P, KC_CTX, UXW), dtype=f16)
        wctx[..., :RANK] = w3
        wctx[..., RANK:] = ctx3
        in_maps.append({
            "ux": ux,
            "wctx": wctx,
            "vt": vt4,
            "consts": consts,
        })
    return in_maps


def _gather_out(results):
    out = np.empty((B_SZ, UNITS), dtype=np.float32)
    for c in range(N_CORES):
        oT = np.asarray(results[c]["outT"])  # [OG, P, 2, BS] fp16
        out[c * BS:(c + 1) * BS, :] = (
            oT.transpose(3, 0, 2, 1).reshape(BS, UNITS).astype(np.float32)
        )
    return out


def kernel(inputs, context, U, S, V, W, B, bias):
    in_maps = _prepare_in_maps(inputs, context, U, S, V, W, B, bias)
    nc = _get_nc()
    res = run_bass_kernel_spmd(nc, in_maps, list(range(N_CORES)))
    return _gather_out(res.results)


# revision 4
# speedup vs baseline: 1.4772x; 1.1395x over previous
"""CASVDDenseMul fused kernel for 8 Trainium2 NeuronCores.

Reference computation (fp32):
    chi = sigmoid(context @ W + B)          # [B, R]
    t   = (inputs @ U) * (S * chi)          # [B, R]
    out = relu(t @ V.T + 2*bias)            # [B, UNITS]

Sharding: data-parallel over batch; each of the 8 cores handles 512 rows.
All factor weights (U, S, V, W, B, bias) are replicated.

The kernel is HBM-bandwidth-bound (~334 GB/s effective per core).
Everything the PE touches is fp16 (PE runs 16-bit at 1 row/cycle, same as
fp32r, and the 2e-2 rel-err budget dwarfs fp16's ~4e-4), which halves HBM
traffic vs the fp32 baseline: ~13.4 MB/core total.

Scheduling (from trace analysis):
  - One DMA ring can saturate HBM by itself, and transfers within a ring
    run strictly in order. So ALL bulk traffic rides the sync ring in the
    exact order we want bytes on the wire: ux pairs -> vt groups -> out
    pairs. vt therefore lands right when mm1 drains, and mm2 starts ~28us
    in instead of ~35.
  - The scalar ring carries only wctx + the two chi sigmoids + half the
    PSUM evacuations, so the sigmoid (and its ACT_TABLE_LOAD) runs at
    ~13us, long before t' needs s_chi.
  - mm2 PSUM tiles are single banks rotating through a 6-buffer pool
    (shared with the warm-up/chi tiles); with only 2 paired buffers the
    matmul<->evac ping-pong serialized the whole output phase.
  - PSUM evacuation is the mm2-phase engine constraint (ACT ~690ns and
    DVE ~690ns per 512-col tile; GPSIMD has no PSUM port), so ACT takes
    even unit-tiles and DVE odd ones, and each evacuated pair shares one
    out DMA (2 KB/partition rows).
"""

import numpy as np

from concourse import bacc, mybir
from concourse import tile
from concourse.bass_utils import run_bass_kernel_spmd

N_CORES = 8
B_SZ, N_IN, N_CTX, UNITS, RANK = 4096, 4096, 512, 4096, 256
BS = B_SZ // N_CORES  # 512 batch rows per core

P = 128
KC_CTX = N_CTX // P      # 4 contraction chunks for ctx @ W
RT = RANK // P           # 2 rank tiles
MT = UNITS // P          # 32 unit (output) tiles
PAIRS = N_IN // (2 * P)  # 16 packed x/U chunk pairs
MG = 8                   # vt groups (512 units each)
MW = UNITS // MG         # 512
OG = MT // 2             # 16 output tile pairs
UXW = RANK + BS          # 768 packed columns per k-chunk

FP32 = mybir.dt.float32
FP16 = mybir.dt.float16


def _build_nc():
    nc = bacc.Bacc("TRN2", target_bir_lowering=False, debug=False, enable_asserts=False)

    ux = nc.declare_dram_parameter("ux", [PAIRS, P, 2, UXW], FP16, isOutput=False)
    wctx = nc.declare_dram_parameter("wctx", [P, KC_CTX, UXW], FP16, isOutput=False)
    vt = nc.declare_dram_parameter("vt", [MG, P, RT, MW], FP16, isOutput=False)
    consts = nc.declare_dram_parameter("consts", [P, 2 * RT + MT], FP32, isOutput=False)
    outT = nc.declare_dram_parameter("outT", [OG, P, 2, BS], FP16, isOutput=True)

    with tile.TileContext(nc) as tc:
        with (
            tc.tile_pool(name="cpool", bufs=1) as cpool,
            tc.tile_pool(name="small", bufs=1) as small,
            tc.tile_pool(name="stream", bufs=1) as stream,
            tc.tile_pool(name="acts", bufs=1) as acts,
            tc.tile_pool(name="ostage", bufs=10) as ostage,
            tc.tile_pool(name="pwork", bufs=6, space="PSUM") as pwork,
            tc.tile_pool(name="pt", bufs=1, space="PSUM") as pt,
        ):
            # ---- constants + small weights ----
            c_sb = cpool.tile([P, 2 * RT + MT], FP32, tag="consts")
            nc.sync.dma_start(c_sb[:], consts[:])
            s2_sb = c_sb[:, 0:RT]
            b2_sb = c_sb[:, RT:2 * RT]
            bias_sb = c_sb[:, 2 * RT:]

            wctx_sb = small.tile([P, KC_CTX, UXW], FP16, tag="wctx")
            nc.scalar.dma_start(wctx_sb[:], wctx[:])

            # ---- PE warm-up: the HAM clock gate keeps the PE at 1.2 GHz
            # until it has been busy ~3.4us. Junk matmuls on a memset tile
            # during the (otherwise idle) DMA prologue flip it to 2.4 GHz
            # before the real stream begins.
            junk = acts.tile([P, BS], FP32, tag="junk")
            nc.gpsimd.memset(junk[:], 0.0)
            warm_ps = pwork.tile([P, BS], FP32, tag="o", name="warm_ps")
            for _ in range(3):
                nc.tensor.matmul(
                    warm_ps[:],
                    junk[:, :P],
                    junk[:],
                    start=True,
                    stop=True,
                    skip_group_check=True,
                )

            # chi.T pre-activation = W.T @ ctxT on the PE right behind the
            # warm-up, then sigmoid(+B) on ACT / *S on DVE. Both queues are
            # otherwise empty until the output phase, so the chi chain (and
            # the lazy ACT_TABLE_LOAD) completes by ~15us, well before t'.
            psum_chi = [pwork.tile([P, BS], FP32, tag="o", name=f"pchi{rt}")
                        for rt in range(RT)]
            s_chi = [acts.tile([P, BS], FP32, tag=f"schi{rt}", name=f"schi{rt}")
                     for rt in range(RT)]
            for rt in range(RT):
                for k in range(KC_CTX):
                    nc.tensor.matmul(
                        psum_chi[rt][:],
                        wctx_sb[:, k, rt * P:(rt + 1) * P],
                        wctx_sb[:, k, RANK:],
                        start=(k == 0),
                        stop=(k == KC_CTX - 1),
                        skip_group_check=True,
                    )
                nc.scalar.activation(
                    s_chi[rt][:], psum_chi[rt][:],
                    mybir.ActivationFunctionType.Sigmoid,
                    bias=b2_sb[:, rt:rt + 1], scale=1.0,
                )
                nc.vector.tensor_scalar_mul(
                    s_chi[rt][:], s_chi[rt][:], s2_sb[:, rt:rt + 1]
                )

            # ---- bulk input stream, all on the sync ring: ux then vt.
            ux_sb = [stream.tile([P, 2, UXW], FP16, tag=f"ux{g}", name=f"ux{g}")
                     for g in range(PAIRS)]
            for g in range(PAIRS):
                nc.sync.dma_start(ux_sb[g][:], ux[g])
            vt_sb = [small.tile([P, RT, MW], FP16, tag=f"vt{g}", name=f"vt{g}")
                     for g in range(MG)]
            for g in range(MG):
                nc.sync.dma_start(vt_sb[g][:], vt[g])

            # ---- t.T = (U.T @ xT): stream the packed u/x pairs ----
            psum_t = [pt.tile([P, BS], FP32, tag=f"t{rt}", name=f"pt{rt}")
                      for rt in range(RT)]
            for g in range(PAIRS):
                for j in range(2):
                    k = 2 * g + j
                    for rt in range(RT):
                        nc.tensor.matmul(
                            psum_t[rt][:],
                            ux_sb[g][:, j, rt * P:(rt + 1) * P],
                            ux_sb[g][:, j, RANK:],
                            start=(k == 0),
                            stop=(k == 2 * PAIRS - 1),
                            skip_group_check=True,
                        )

            # t' = t * (S*chi), cast to fp16 for mm2
            t_sb = [acts.tile([P, BS], FP16, tag=f"t{rt}", name=f"t_sb{rt}")
                    for rt in range(RT)]
            for rt in range(RT):
                nc.vector.tensor_mul(t_sb[rt][:], psum_t[rt][:], s_chi[rt][:])

            # ---- out.T = relu(V @ t.T + 2*bias): one PSUM bank per
            # 128-unit tile rotating through pwork; ACT evacuates even
            # tiles, DVE odd ones; one out DMA per evacuated pair.
            for og in range(OG):
                osb = ostage.tile([P, 2, BS], FP16, tag="osb", name=f"osb{og}")
                for j in range(2):
                    m = 2 * og + j
                    vg, off = divmod(m, MT // MG)
                    po = pwork.tile([P, BS], FP32, tag="o", name=f"po{m}")
                    for c in range(RT):
                        nc.tensor.matmul(
                            po[:],
                            vt_sb[vg][:, c, off * P:(off + 1) * P],
                            t_sb[c][:],
                            start=(c == 0),
                            stop=(c == RT - 1),
                            skip_group_check=True,
                        )
                    if j == 0:
                        nc.scalar.activation(
                            osb[:, 0, :], po[:],
                            mybir.ActivationFunctionType.Relu,
                            bias=bias_sb[:, m:m + 1], scale=1.0,
                        )
                    else:
                        nc.vector.tensor_scalar(
                            osb[:, 1, :], po[:],
                            bias_sb[:, m:m + 1], 0.0,
                            op0=mybir.AluOpType.add, op1=mybir.AluOpType.max,
                        )
                nc.sync.dma_start(outT[og], osb[:])

    nc.finalize()
    return nc


_NC_CACHE = {}


def _get_nc():
    if "nc" not in _NC_CACHE:
        _NC_CACHE["nc"] = _build_nc()
    return _NC_CACHE["nc"]


def _prepare_in_maps(inputs, context, U, S, V, W, B, bias):
    f16 = np.float16
    xT = np.ascontiguousarray(np.asarray(inputs, dtype=np.float32).T).astype(f16)
    ctxT = np.ascontiguousarray(np.asarray(context, dtype=np.float32).T).astype(f16)

    # U pairs, shared: [PAIRS, P, 2, RANK]
    u4 = np.asarray(U, dtype=np.float32).astype(f16) \
        .reshape(PAIRS, 2, P, RANK).transpose(0, 2, 1, 3)
    # W chunks, shared: [P, KC_CTX, RANK]
    w3 = np.asarray(W, dtype=np.float32).astype(f16) \
        .reshape(KC_CTX, P, RANK).transpose(1, 0, 2)
    # V.T groups, shared: [MG, P, RT, MW]
    vt4 = np.ascontiguousarray(np.asarray(V, dtype=np.float32).T).astype(f16) \
        .reshape(RT, P, MG, MW).transpose(2, 1, 0, 3)
    vt4 = np.ascontiguousarray(vt4)

    S2 = np.asarray(S, dtype=np.float32).reshape(RT, P).T
    B2 = np.asarray(B, dtype=np.float32).reshape(RT, P).T
    bias2 = (2.0 * np.asarray(bias, dtype=np.float32)).reshape(MT, P).T
    consts = np.ascontiguousarray(np.concatenate([S2, B2, bias2], axis=1))

    in_maps = []
    for c in range(N_CORES):
        sl = slice(c * BS, (c + 1) * BS)
        x4 = xT[:, sl].reshape(PAIRS, 2, P, BS).transpose(0, 2, 1, 3)
        ux = np.empty((PAIRS, P, 2, UXW), dtype=f16)
        ux[..., :RANK] = u4
        ux[..., RANK:] = x4
        ctx3 = ctxT[:, sl].reshape(KC_CTX, P, BS).transpose(1, 0, 2)
        wctx = np.empty((P, KC_CTX, UXW), dtype=f16)
        wctx[..., :RANK] = w3
        wctx[..., RANK:] = ctx3
        in_maps.append({
            "ux": ux,
            "wctx": wctx,
            "vt": vt4,
            "consts": consts,
        })
    return in_maps


def _gather_out(results):
    out = np.empty((B_SZ, UNITS), dtype=np.float32)
    for c in range(N_CORES):
        oT = np.asarray(results[c]["outT"])  # [OG, P, 2, BS] fp16
        out[c * BS:(c + 1) * BS, :] = (
            oT.transpose(3, 0, 2, 1).reshape(BS, UNITS).astype(np.float32)
        )
    return out


def kernel(inputs, context, U, S, V, W, B, bias):
    in_maps = _prepare_in_maps(inputs, context, U, S, V, W, B, bias)
    nc = _get_nc()
    res = run_bass_kernel_spmd(nc, in_maps, list(range(N_CORES)))
    return _gather_out(res.results)


# revision 6
# speedup vs baseline: 1.6814x; 1.1382x over previous
"""CASVDDenseMul fused kernel for 8 Trainium2 NeuronCores.

Reference computation (fp32):
    chi = sigmoid(context @ W + B)          # [B, R]
    t   = (inputs @ U) * (S * chi)          # [B, R]
    out = relu(t @ V.T + 2*bias)            # [B, UNITS]

Sharding: data-parallel over batch; each of the 8 cores handles 512 rows.
All factor weights (U, S, V, W, B, bias) are replicated.

The kernel is HBM-bandwidth-bound (~334 GB/s effective per core).
Everything the PE touches is fp16 (PE runs 16-bit at 1 row/cycle, same as
fp32r, and the 2e-2 rel-err budget dwarfs fp16's ~4e-4), which halves HBM
traffic vs the fp32 baseline: ~13.4 MB/core total.

Scheduling (from trace analysis):
  - One DMA ring can saturate HBM by itself, and transfers within a ring
    run strictly in order. So ALL bulk traffic rides the sync ring in the
    exact order we want bytes on the wire: ux pairs -> vt groups -> out
    pairs. vt therefore lands right when mm1 drains, and mm2 starts ~28us
    in instead of ~35.
  - The scalar ring carries only wctx + the two chi sigmoids + half the
    PSUM evacuations, so the sigmoid (and its ACT_TABLE_LOAD) runs at
    ~13us, long before t' needs s_chi.
  - mm2 PSUM tiles are single banks rotating through a 6-buffer pool
    (shared with the warm-up/chi tiles); with only 2 paired buffers the
    matmul<->evac ping-pong serialized the whole output phase.
  - PSUM evacuation is the mm2-phase engine constraint (ACT ~690ns and
    DVE ~690ns per 512-col tile; GPSIMD has no PSUM port), so ACT takes
    even unit-tiles and DVE odd ones, and each evacuated pair shares one
    out DMA (2 KB/partition rows).
"""

import numpy as np

from concourse import bacc, mybir
from concourse import tile
from concourse.bass_utils import run_bass_kernel_spmd

N_CORES = 8
B_SZ, N_IN, N_CTX, UNITS, RANK = 4096, 4096, 512, 4096, 256
BS = B_SZ // N_CORES  # 512 batch rows per core

P = 128
KC_CTX = N_CTX // P      # 4 contraction chunks for ctx @ W
RT = RANK // P           # 2 rank tiles
MT = UNITS // P          # 32 unit (output) tiles
PAIRS = N_IN // (2 * P)  # 16 packed x/U chunk pairs
MG = 8                   # vt groups (512 units each)
MW = UNITS // MG         # 512
OG = MT // 2             # 16 output tile pairs
UXW = RANK + BS          # 768 packed columns per k-chunk

FP32 = mybir.dt.float32
FP16 = mybir.dt.float16


def _build_nc():
    nc = bacc.Bacc("TRN2", target_bir_lowering=False, debug=False, enable_asserts=False)

    ux = nc.declare_dram_parameter("ux", [PAIRS, P, 2, UXW], FP16, isOutput=False)
    wctx = nc.declare_dram_parameter("wctx", [P, KC_CTX, UXW], FP16, isOutput=False)
    vt = nc.declare_dram_parameter("vt", [MG, P, RT, MW], FP16, isOutput=False)
    consts = nc.declare_dram_parameter("consts", [P, 2 * RT + MT], FP32, isOutput=False)
    outT = nc.declare_dram_parameter("outT", [OG, P, 2, BS], FP16, isOutput=True)

    with tile.TileContext(nc) as tc:
        with (
            tc.tile_pool(name="cpool", bufs=1) as cpool,
            tc.tile_pool(name="small", bufs=1) as small,
            tc.tile_pool(name="stream", bufs=1) as stream,
            tc.tile_pool(name="acts", bufs=1) as acts,
            tc.tile_pool(name="ostage", bufs=10) as ostage,
            tc.tile_pool(name="pwork", bufs=6, space="PSUM") as pwork,
            tc.tile_pool(name="pt", bufs=1, space="PSUM") as pt,
        ):
            # ---- constants + small weights ----
            c_sb = cpool.tile([P, 2 * RT + MT], FP32, tag="consts")
            nc.sync.dma_start(c_sb[:], consts[:])
            s2_sb = c_sb[:, 0:RT]
            b2_sb = c_sb[:, RT:2 * RT]
            bias_sb = c_sb[:, 2 * RT:]

            # wctx arrives as per-chunk DMAs so chi's first matmuls can start
            # on chunk 0 (~9.6us) instead of waiting for the full tensor.
            wctx_sb = small.tile([P, KC_CTX, UXW], FP16, tag="wctx")
            for k in range(KC_CTX):
                nc.scalar.dma_start(wctx_sb[:, k, :], wctx[:, k, :])

            # ---- PE warm-up: the HAM clock gate keeps the PE at 1.2 GHz
            # until it has been busy ~3.4us. Junk matmuls on a memset tile
            # during the (otherwise idle) DMA prologue flip it to 2.4 GHz
            # before the real stream begins.
            junk = acts.tile([P, BS], FP32, tag="junk")
            nc.gpsimd.memset(junk[:], 0.0)
            warm_ps = pwork.tile([P, BS], FP32, tag="o", name="warm_ps")
            for _ in range(3):
                nc.tensor.matmul(
                    warm_ps[:],
                    junk[:, :P],
                    junk[:],
                    start=True,
                    stop=True,
                    skip_group_check=True,
                )

            # ---- bulk input stream, all on the sync ring: ux then vt.
            ux_sb = [stream.tile([P, 2, UXW], FP16, tag=f"ux{g}", name=f"ux{g}")
                     for g in range(PAIRS)]
            for g in range(PAIRS):
                nc.sync.dma_start(ux_sb[g][:], ux[g])
            vt_sb = [small.tile([P, RT, MW], FP16, tag=f"vt{g}", name=f"vt{g}")
                     for g in range(MG)]
            for g in range(MG):
                nc.sync.dma_start(vt_sb[g][:], vt[g])

            # chi epilogue targets: sigmoid(+B) on ACT, *S on DVE. Queued
            # early; both engines are idle until the output phase.
            psum_chi = [pwork.tile([P, BS], FP32, tag="o", name=f"pchi{rt}")
                        for rt in range(RT)]
            s_chi = [acts.tile([P, BS], FP32, tag=f"schi{rt}", name=f"schi{rt}")
                     for rt in range(RT)]

            def emit_chi_chunk(k):
                # one ctx@W contraction chunk for both rank tiles; chunk 0
                # opens the accumulation, chunk 3 closes it and chains the
                # sigmoid + S-multiply.
                for rt in range(RT):
                    nc.tensor.matmul(
                        psum_chi[rt][:],
                        wctx_sb[:, k, rt * P:(rt + 1) * P],
                        wctx_sb[:, k, RANK:],
                        start=(k == 0),
                        stop=(k == KC_CTX - 1),
                        skip_group_check=True,
                    )
                if k == KC_CTX - 1:
                    for rt in range(RT):
                        nc.scalar.activation(
                            s_chi[rt][:], psum_chi[rt][:],
                            mybir.ActivationFunctionType.Sigmoid,
                            bias=b2_sb[:, rt:rt + 1], scale=1.0,
                        )
                        nc.vector.tensor_scalar_mul(
                            s_chi[rt][:], s_chi[rt][:], s2_sb[:, rt:rt + 1]
                        )

            # ---- t.T = (U.T @ xT): stream the packed u/x pairs. The PE is
            # the critical resource end-to-end (sustained load drops the
            # core clock to ~5/6 nominal), so chi's 8 matmuls are laced into
            # the first four mm1 pairs where the DMA pacing leaves ~140ns of
            # PE slack per pair, instead of running serially before mm1.
            psum_t = [pt.tile([P, BS], FP32, tag=f"t{rt}", name=f"pt{rt}")
                      for rt in range(RT)]
            for g in range(PAIRS):
                if g < KC_CTX:
                    emit_chi_chunk(g)
                for j in range(2):
                    k = 2 * g + j
                    for rt in range(RT):
                        nc.tensor.matmul(
                            psum_t[rt][:],
                            ux_sb[g][:, j, rt * P:(rt + 1) * P],
                            ux_sb[g][:, j, RANK:],
                            start=(k == 0),
                            stop=(k == 2 * PAIRS - 1),
                            skip_group_check=True,
                        )

            # t' = t * (S*chi), cast to fp16 for mm2
            t_sb = [acts.tile([P, BS], FP16, tag=f"t{rt}", name=f"t_sb{rt}")
                    for rt in range(RT)]
            for rt in range(RT):
                nc.vector.tensor_mul(t_sb[rt][:], psum_t[rt][:], s_chi[rt][:])

            # ---- out.T = relu(V @ t.T + 2*bias): one PSUM bank per
            # 128-unit tile rotating through pwork; ACT evacuates even
            # tiles, DVE odd ones; one out DMA per evacuated pair.
            for og in range(OG):
                osb = ostage.tile([P, 2, BS], FP16, tag="osb", name=f"osb{og}")
                for j in range(2):
                    m = 2 * og + j
                    vg, off = divmod(m, MT // MG)
                    po = pwork.tile([P, BS], FP32, tag="o", name=f"po{m}")
                    for c in range(RT):
                        nc.tensor.matmul(
                            po[:],
                            vt_sb[vg][:, c, off * P:(off + 1) * P],
                            t_sb[c][:],
                            start=(c == 0),
                            stop=(c == RT - 1),
                            skip_group_check=True,
                        )
                    if j == 0:
                        nc.scalar.activation(
                            osb[:, 0, :], po[:],
                            mybir.ActivationFunctionType.Relu,
                            bias=bias_sb[:, m:m + 1], scale=1.0,
                        )
                    else:
                        nc.vector.tensor_scalar(
                            osb[:, 1, :], po[:],
                            bias_sb[:, m:m + 1], 0.0,
                            op0=mybir.AluOpType.add, op1=mybir.AluOpType.max,
                        )
                nc.sync.dma_start(outT[og], osb[:])

    nc.finalize()
    return nc


_NC_CACHE = {}


def _get_nc():
    if "nc" not in _NC_CACHE:
        _NC_CACHE["nc"] = _build_nc()
    return _NC_CACHE["nc"]


def _prepare_in_maps(inputs, context, U, S, V, W, B, bias):
    f16 = np.float16
    xT = np.ascontiguousarray(np.asarray(inputs, dtype=np.float32).T).astype(f16)
    ctxT = np.ascontiguousarray(np.asarray(context, dtype=np.float32).T).astype(f16)

    # U pairs, shared: [PAIRS, P, 2, RANK]
    u4 = np.asarray(U, dtype=np.float32).astype(f16) \
        .reshape(PAIRS, 2, P, RANK).transpose(0, 2, 1, 3)
    # W chunks, shared: [P, KC_CTX, RANK]
    w3 = np.asarray(W, dtype=np.float32).astype(f16) \
        .reshape(KC_CTX, P, RANK).transpose(1, 0, 2)
    # V.T groups, shared: [MG, P, RT, MW]
    vt4 = np.ascontiguousarray(np.asarray(V, dtype=np.float32).T).astype(f16) \
        .reshape(RT, P, MG, MW).transpose(2, 1, 0, 3)
    vt4 = np.ascontiguousarray(vt4)

    S2 = np.asarray(S, dtype=np.float32).reshape(RT, P).T
    B2 = np.asarray(B, dtype=np.float32).reshape(RT, P).T
    bias2 = (2.0 * np.asarray(bias, dtype=np.float32)).reshape(MT, P).T
    consts = np.ascontiguousarray(np.concatenate([S2, B2, bias2], axis=1))

    in_maps = []
    for c in range(N_CORES):
        sl = slice(c * BS, (c + 1) * BS)
        x4 = xT[:, sl].reshape(PAIRS, 2, P, BS).transpose(0, 2, 1, 3)
        ux = np.empty((PAIRS, P, 2, UXW), dtype=f16)
        ux[..., :RANK] = u4
        ux[..., RANK:] = x4
        ctx3 = ctxT[:, sl].reshape(KC_CTX, P, BS).transpose(1, 0, 2)
        wctx = np.empty((P, KC_CTX, UXW), dtype=f16)
        wctx[..., :RANK] = w3
        wctx[..., RANK:] = ctx3
        in_maps.append({
            "ux": ux,
            "wctx": wctx,
            "vt": vt4,
            "consts": consts,
        })
    return in_maps


def _gather_out(results):
    out = np.empty((B_SZ, UNITS), dtype=np.float32)
    for c in range(N_CORES):
        oT = np.asarray(results[c]["outT"])  # [OG, P, 2, BS] fp16
        out[c * BS:(c + 1) * BS, :] = (
            oT.transpose(3, 0, 2, 1).reshape(BS, UNITS).astype(np.float32)
        )
    return out


def kernel(inputs, context, U, S, V, W, B, bias):
    in_maps = _prepare_in_maps(inputs, context, U, S, V, W, B, bias)
    nc = _get_nc()
    res = run_bass_kernel_spmd(nc, in_maps, list(range(N_CORES)))
    return _gather_out(res.results)
